# revision 2
# baseline (speedup 1.0000x reference)
"""Trainium2 Bass kernel for nn_CustomModel_13657996001613 (moe_routing).

Distribution: data-parallel over nodes (1024 nodes -> 8 cores x 128).
The two GCN segment-sums use host-bucketed edges (by dst-shard, src-shard),
one-hot outer-product matmuls to build 128x128 normalized-adjacency blocks,
and an AllGather of the (dinv-scaled) per-shard features.
The l-reductions of wm1/class2/wm2 are sharded over k across cores and
combined with one tiny packed AllGather.
"""
import sys

if "/opt/trn_rl_repo" not in sys.path:
    sys.path.insert(0, "/opt/trn_rl_repo")

import numpy as np

import concourse.bass as bass  # noqa: F401
import concourse.mybir as mybir
import concourse.tile as tile
from concourse import bacc, bass_utils
from concourse.masks import make_identity

F32 = mybir.dt.float32
OP = mybir.AluOpType
AF = mybir.ActivationFunctionType
AX = mybir.AxisListType

NCORE = 8
P = 128          # nodes per core == SBUF partitions
NNODE = 1024
D = 768
KX = D // P      # 6
H = 512
HT = H // P      # 4
F1 = 256
F2 = 32
TAU = 0.7
RLO = (1.0 - TAU) / 2.0       # 0.15
RDELT = TAU - RLO             # 0.55

_module_cache: dict[int, "bacc.Bacc"] = {}


def _build(T: int) -> "bacc.Bacc":
    nc = bacc.Bacc(
        "TRN2",
        target_bir_lowering=False,
        debug=False,
        enable_asserts=False,
        num_devices=NCORE,
    )
    dt = F32

    def dram(name, shape, kind="ExternalInput"):
        return nc.dram_tensor(name, list(shape), dt, kind=kind)

    x1t_d = dram("x1t", [KX, P, P])
    x11t_d = dram("x11t", [KX, P, P])
    x2t_d = dram("x2t", [KX, P, P])
    mW1_d = dram("mW1", [KX, P, H])
    mW2_d = dram("mW2", [HT, P, H])
    mW3_d = dram("mW3", [HT, P, 3])
    mb1_d = dram("mb1", [P, HT])
    mb2_d = dram("mb2", [P, HT])
    mb3_d = dram("mb3", [P, 3])
    gW1_d = dram("gW1", [KX, P, H])
    gW2_d = dram("gW2", [HT, P, H])
    gW3_d = dram("gW3", [HT, P, 2])
    gb1_d = dram("gb1", [P, HT])
    gb2_d = dram("gb2", [P, HT])
    gb3_d = dram("gb3", [2, 1])
    bp1_d = dram("bp1", [2, D])
    bp2_d = dram("bp2", [3, D])
    w1p_d = dram("w1p", [KX, P, 96])    # wm1[:, kslice, :] as [j*l, k] tiles
    c2p_d = dram("c2p", [2, 64, 96])    # class2[:, kslice, :] as [j][l, k]
    w2p_d = dram("w2p", [KX, P, 32])    # wm2[:, kslice, :] as [j*l, k] tiles
    wm12_d = dram("wm12", [3, F1])
    w13_d = dram("w13", [KX, P, F1])
    g1W_d = dram("g1W", [KX, P, F1])
    g1b_d = dram("g1b", [P, F1])
    g2W_d = dram("g2W", [2, P, F2])
    g2b_d = dram("g2b", [P, F2])
    fcW_d = dram("fcW", [F2, 8])
    fcb_d = dram("fcb", [P, 8])
    esrc_d = dram("esrc", [P, NCORE * T])
    edst_d = dram("edst", [P, NCORE * T])
    out_d = dram("out", [P, 8], kind="ExternalOutput")

    with tile.TileContext(nc) as tc:
        from contextlib import ExitStack

        ctx = ExitStack()
        with ctx:
            const = ctx.enter_context(tc.tile_pool(name="const", bufs=1))
            wbig = ctx.enter_context(tc.tile_pool(name="wbig", bufs=3))
            wmid = ctx.enter_context(tc.tile_pool(name="wmid", bufs=3))
            wsml = ctx.enter_context(tc.tile_pool(name="wsml", bufs=2))
            xpool = ctx.enter_context(tc.tile_pool(name="xpool", bufs=1))
            actp = ctx.enter_context(tc.tile_pool(name="actp", bufs=9))
            resp = ctx.enter_context(tc.tile_pool(name="resp", bufs=1))
            ohp = ctx.enter_context(tc.tile_pool(name="ohp", bufs=4))
            apool = ctx.enter_context(tc.tile_pool(name="apool", bufs=1))
            hallp = ctx.enter_context(tc.tile_pool(name="hallp", bufs=1))
            work = ctx.enter_context(tc.tile_pool(name="work", bufs=1))
            ps = ctx.enter_context(
                tc.tile_pool(name="ps", bufs=8, space="PSUM")
            )
            dpool = ctx.enter_context(
                tc.tile_pool(name="dram", bufs=1, space="DRAM")
            )

            # ---- constants --------------------------------------------------
            iota_sb = const.tile([P, P], dt, tag="iota", name="iota")
            nc.gpsimd.iota(
                iota_sb[:],
                pattern=[[1, P]],
                base=0,
                channel_multiplier=0,
                allow_small_or_imprecise_dtypes=True,
            )
            ident = const.tile([P, P], dt, tag="ident", name="ident")
            make_identity(nc, ident[:])
            ones_sb = const.tile([P, 1], dt, tag="ones", name="ones")
            nc.vector.memset(ones_sb[:], 1.0)

            # ---- small inputs ----------------------------------------------
            def load(pool, shape, dsrc, tag):
                t = pool.tile(list(shape), dt, tag=tag)
                nc.sync.dma_start(t[:], dsrc)
                return t

            mb1_sb = load(const, [P, HT], mb1_d[:, :], "mb1")
            mb2_sb = load(const, [P, HT], mb2_d[:, :], "mb2")
            mb3_sb = load(const, [P, 3], mb3_d[:, :], "mb3")
            gb1_sb = load(const, [P, HT], gb1_d[:, :], "gb1")
            gb2_sb = load(const, [P, HT], gb2_d[:, :], "gb2")
            gb3_sb = load(const, [2, 1], gb3_d[:, :], "gb3")
            bp1_sb = load(const, [2, D], bp1_d[:, :], "bp1")
            bp2_sb = load(const, [3, D], bp2_d[:, :], "bp2")
            wm12_sb = load(const, [3, F1], wm12_d[:, :], "wm12")
            g1b_sb = load(const, [P, F1], g1b_d[:, :], "g1b")
            g2b_sb = load(const, [P, F2], g2b_d[:, :], "g2b")
            fcW_sb = load(const, [F2, 8], fcW_d[:, :], "fcW")
            fcb_sb = load(const, [P, 8], fcb_d[:, :], "fcb")
            esrc_sb = load(const, [P, NCORE * T], esrc_d[:, :], "esrc")
            edst_sb = load(const, [P, NCORE * T], edst_d[:, :], "edst")

            x1_sb = [load(xpool, [P, P], x1t_d[k], f"x1_{k}") for k in range(KX)]
            x11_sb = [load(xpool, [P, P], x11t_d[k], f"x11_{k}") for k in range(KX)]
            x2_sb = [load(xpool, [P, P], x2t_d[k], f"x2_{k}") for k in range(KX)]

            # ---- sharded l-reductions of wm1 / class2 / wm2 + tiny AG ------
            agg_sb = work.tile([1, 576], dt, tag="agg", name="agg")
            for j in range(3):
                psr = ps.tile([1, 96], dt, tag="ps", name="ps")
                for lt in range(2):
                    wt = wsml.tile([P, 96], dt, tag="w96", name="w96")
                    nc.sync.dma_start(wt[:], w1p_d[2 * j + lt])
                    nc.tensor.matmul(
                        psr[:], ones_sb[:, 0:1], wt[:],
                        start=(lt == 0), stop=(lt == 1),
                    )
                nc.vector.tensor_copy(agg_sb[0:1, j * 96:(j + 1) * 96], psr[:])
            for j in range(2):
                c2t = wsml.tile([64, 96], dt, tag="c2", name="c2")
                nc.sync.dma_start(c2t[:], c2p_d[j])
                psr = ps.tile([1, 96], dt, tag="ps", name="ps")
                nc.tensor.matmul(psr[:], ones_sb[0:64, 0:1], c2t[:])
                nc.vector.tensor_copy(
                    agg_sb[0:1, 288 + j * 96:288 + (j + 1) * 96], psr[:]
                )
            for j in range(3):
                psr = ps.tile([1, 32], dt, tag="ps", name="ps")
                for lt in range(2):
                    wt = wsml.tile([P, 32], dt, tag="w32", name="w32")
                    nc.sync.dma_start(wt[:], w2p_d[2 * j + lt])
                    nc.tensor.matmul(
                        psr[:], ones_sb[:, 0:1], wt[:],
                        start=(lt == 0), stop=(lt == 1),
                    )
                nc.vector.tensor_copy(
                    agg_sb[0:1, 480 + j * 32:480 + (j + 1) * 32], psr[:]
                )
            aggin_t = dpool.tile([1, 576], dt, tag="aggin", name="aggin")
            aggout_t = dpool.tile([NCORE, 576], dt, tag="aggout", name="aggout")
            nc.sync.dma_start(aggin_t[:], agg_sb[:])
            nc.gpsimd.collective_compute(
                "AllGather",
                OP.bypass,
                replica_groups=[list(range(NCORE))],
                ins=[aggin_t[:].opt()],
                outs=[aggout_t[:].opt()],
            )
            W1s_sb = const.tile([3, D], dt, tag="W1s", name="W1s")
            C2s_sb = const.tile([2, D], dt, tag="C2s", name="C2s")
            W2s_sb = const.tile([3, F1], dt, tag="W2s", name="W2s")
            ag_ap = aggout_t[:]
            nc.sync.dma_start(
                W1s_sb[:].rearrange("j (c t) -> j c t", c=NCORE),
                ag_ap[:, 0:288].rearrange("c (j t) -> j c t", j=3),
            )
            nc.sync.dma_start(
                C2s_sb[:].rearrange("j (c t) -> j c t", c=NCORE),
                ag_ap[:, 288:480].rearrange("c (j t) -> j c t", j=2),
            )
            nc.sync.dma_start(
                W2s_sb[:].rearrange("j (c t) -> j c t", c=NCORE),
                ag_ap[:, 480:576].rearrange("c (j t) -> j c t", j=3),
            )

            # ---- MLP branches ----------------------------------------------
            def mlp_2layers(x_sb, W1d, W2d, b1_sb, b2_sb, pfx):
                ps_m = [ps.tile([P, P], dt, tag="ps", name="ps") for _ in range(HT)]
                for k in range(KX):
                    w = wbig.tile([P, H], dt, tag="wbig", name="wbig")
                    nc.sync.dma_start(w[:], W1d[k])
                    for m in range(HT):
                        nc.tensor.matmul(
                            ps_m[m][:], w[:, m * P:(m + 1) * P], x_sb[k][:],
                            start=(k == 0), stop=(k == KX - 1),
                        )
                h1 = []
                for m in range(HT):
                    t = actp.tile([P, P], dt, tag="ht", name="ht")
                    nc.scalar.activation(
                        t[:], ps_m[m][:], AF.Relu, bias=b1_sb[:, m:m + 1]
                    )
                    h1.append(t)
                ps_m2 = [ps.tile([P, P], dt, tag="ps", name="ps") for _ in range(HT)]
                for k in range(HT):
                    w = wbig.tile([P, H], dt, tag="wbig", name="wbig")
                    nc.sync.dma_start(w[:], W2d[k])
                    for m in range(HT):
                        nc.tensor.matmul(
                            ps_m2[m][:], w[:, m * P:(m + 1) * P], h1[k][:],
                            start=(k == 0), stop=(k == HT - 1),
                        )
                h2 = []
                for m in range(HT):
                    t = actp.tile([P, P], dt, tag="ht", name="ht")
                    nc.scalar.activation(
                        t[:], ps_m2[m][:], AF.Relu, bias=b2_sb[:, m:m + 1]
                    )
                    h2.append(t)
                return h2

            # branch g (x11 -> gT [2,128] feature-major)
            g_h2 = mlp_2layers(x11_sb, gW1_d, gW2_d, gb1_sb, gb2_sb, "g")
            gW3_sb = [load(wsml, [P, 2], gW3_d[k], "gW3") for k in range(HT)]
            ps_g = ps.tile([2, P], dt, tag="ps", name="ps")
            for k in range(HT):
                nc.tensor.matmul(
                    ps_g[:], gW3_sb[k][:], g_h2[k][:],
                    start=(k == 0), stop=(k == HT - 1),
                )
            gT_sb = work.tile([2, P], dt, tag="gT", name="gT")
            nc.scalar.activation(
                gT_sb[:], ps_g[:], AF.Identity, bias=gb3_sb[:, 0:1]
            )

            # branch logits (x1 -> r [128,3] node-major -> rT [3,128])
            m_h2 = mlp_2layers(x1_sb, mW1_d, mW2_d, mb1_sb, mb2_sb, "m")
            mW3_sb = [load(wsml, [P, 3], mW3_d[k], "mW3") for k in range(HT)]
            ps_l = ps.tile([P, 3], dt, tag="ps", name="ps")
            for k in range(HT):
                nc.tensor.matmul(
                    ps_l[:], m_h2[k][:], mW3_sb[k][:],
                    start=(k == 0), stop=(k == HT - 1),
                )
            logits_sb = work.tile([P, 3], dt, tag="logits", name="logits")
            nc.vector.tensor_tensor(logits_sb[:], ps_l[:], mb3_sb[:], op=OP.add)
            mx_sb = work.tile([P, 1], dt, tag="mx", name="mx")
            nc.vector.tensor_reduce(mx_sb[:], logits_sb[:], axis=AX.X, op=OP.max)
            r_sb = work.tile([P, 3], dt, tag="r", name="r")
            nc.vector.tensor_scalar(
                r_sb[:], logits_sb[:], mx_sb[:, 0:1], None, OP.is_ge
            )
            nc.vector.tensor_scalar(
                r_sb[:], r_sb[:], RDELT, RLO, OP.mult, OP.add
            )
            ps_rt = ps.tile([3, P], dt, tag="ps", name="ps")
            nc.tensor.transpose(ps_rt[:], r_sb[:], ident[:])
            rT_sb = work.tile([3, P], dt, tag="rT", name="rT")
            nc.vector.tensor_copy(rT_sb[:], ps_rt[:])

            # ---- edge one-hots -> A_T blocks + deg + dinv -------------------
            A_sb = []
            for s in range(NCORE):
                psA = ps.tile([P, P], dt, tag="ps", name="ps")
                for t_ in range(T):
                    col = s * T + t_
                    sd = ohp.tile([P, P], dt, tag="oh", name="oh")
                    nc.vector.tensor_scalar(
                        sd[:], iota_sb[:], edst_sb[:, col:col + 1], None,
                        OP.is_equal,
                    )
                    ss = ohp.tile([P, P], dt, tag="oh", name="oh")
                    nc.vector.tensor_scalar(
                        ss[:], iota_sb[:], esrc_sb[:, col:col + 1], None,
                        OP.is_equal,
                    )
                    nc.tensor.matmul(
                        psA[:], ss[:], sd[:],
                        start=(t_ == 0), stop=(t_ == T - 1),
                    )
                asb = apool.tile([P, P], dt, tag=f"A{s}", name=f"A{s}")
                nc.vector.tensor_copy(asb[:], psA[:])
                A_sb.append(asb)
            ps_deg = ps.tile([P, 1], dt, tag="ps", name="ps")
            for s in range(NCORE):
                nc.tensor.matmul(
                    ps_deg[:], A_sb[s][:], ones_sb[:, 0:1],
                    start=(s == 0), stop=(s == NCORE - 1),
                )
            dmax_sb = work.tile([P, 1], dt, tag="dmax", name="dmax")
            nc.vector.tensor_scalar_max(dmax_sb[:], ps_deg[:], 1.0)
            dsq_sb = work.tile([P, 1], dt, tag="dsq", name="dsq")
            nc.scalar.activation(dsq_sb[:], dmax_sb[:], AF.Sqrt)
            drec_sb = work.tile([P, 1], dt, tag="drec", name="drec")
            nc.vector.reciprocal(drec_sb[:], dsq_sb[:])
            mask_sb = work.tile([P, 1], dt, tag="mask", name="mask")
            nc.vector.tensor_scalar(
                mask_sb[:], ps_deg[:], 0.0, None, OP.is_gt
            )
            dinv_sb = work.tile([P, 1], dt, tag="dinv", name="dinv")
            nc.vector.tensor_tensor(
                dinv_sb[:], drec_sb[:], mask_sb[:], op=OP.mult
            )

            # ---- res1 chain (feature-major [768,128] in 6 tiles) ------------
            res1_sb = []
            for k in range(KX):
                ksl = slice(k * P, (k + 1) * P)
                ps_P = ps.tile([P, P], dt, tag="ps", name="ps")
                nc.tensor.matmul(ps_P[:], C2s_sb[:, ksl], gT_sb[:])
                ps_Q = ps.tile([P, P], dt, tag="ps", name="ps")
                nc.tensor.matmul(ps_Q[:], bp1_sb[:, ksl], gT_sb[:])
                r1 = resp.tile([P, P], dt, tag=f"res1_{k}", name=f"res1_{k}")
                nc.vector.tensor_tensor(r1[:], ps_P[:], x2_sb[k][:], op=OP.mult)
                nc.vector.tensor_tensor(r1[:], r1[:], ps_Q[:], op=OP.add)
                ps_P2 = ps.tile([P, P], dt, tag="ps", name="ps")
                nc.tensor.matmul(ps_P2[:], W1s_sb[:, ksl], rT_sb[:])
                ps_Q2 = ps.tile([P, P], dt, tag="ps", name="ps")
                nc.tensor.matmul(ps_Q2[:], bp2_sb[:, ksl], rT_sb[:])
                nc.vector.tensor_tensor(r1[:], r1[:], ps_P2[:], op=OP.mult)
                nc.vector.tensor_tensor(r1[:], r1[:], ps_Q2[:], op=OP.add)
                res1_sb.append(r1)

            # ---- GCN1 -------------------------------------------------------
            ps_h = ps.tile([P, F1], dt, tag="ps", name="ps")
            for k in range(KX):
                gw = wmid.tile([P, F1], dt, tag="wmid", name="wmid")
                nc.sync.dma_start(gw[:], g1W_d[k])
                nc.tensor.matmul(
                    ps_h[:], res1_sb[k][:], gw[:],
                    start=(k == 0), stop=(k == KX - 1),
                )
            h1g_sb = work.tile([P, F1], dt, tag="h1g", name="h1g")
            nc.scalar.activation(
                h1g_sb[:], ps_h[:], AF.Copy, bias=0.0, scale=dinv_sb[:, 0:1]
            )
            cin1_t = dpool.tile([P, F1], dt, tag="cin1", name="cin1")
            cout1_t = dpool.tile([NNODE, F1], dt, tag="cout1", name="cout1")
            nc.sync.dma_start(cin1_t[:], h1g_sb[:])
            nc.gpsimd.collective_compute(
                "AllGather",
                OP.bypass,
                replica_groups=[list(range(NCORE))],
                ins=[cin1_t[:].opt()],
                outs=[cout1_t[:].opt()],
            )
            hall = []
            for s in range(NCORE):
                t = hallp.tile([P, F1], dt, tag=f"hall{s}", name=f"hall{s}")
                nc.sync.dma_start(t[:], cout1_t[:][s * P:(s + 1) * P, :])
                hall.append(t)
            ps_o1 = ps.tile([P, F1], dt, tag="ps", name="ps")
            for s in range(NCORE):
                nc.tensor.matmul(
                    ps_o1[:], A_sb[s][:], hall[s][:],
                    start=(s == 0), stop=(s == NCORE - 1),
                )
            h1c_sb = work.tile([P, F1], dt, tag="h1c", name="h1c")
            nc.vector.scalar_tensor_tensor(
                h1c_sb[:], ps_o1[:], dinv_sb[:, 0:1], g1b_sb[:],
                op0=OP.mult, op1=OP.add,
            )
            h1r_sb = work.tile([P, F1], dt, tag="h1r", name="h1r")
            nc.scalar.activation(h1r_sb[:], h1c_sb[:], AF.Relu)

            # ---- h1 post: (r@wm12)*h1 + 2e-4*(res1@wm13) --------------------
            ps_rw = ps.tile([P, F1], dt, tag="ps", name="ps")
            nc.tensor.matmul(ps_rw[:], rT_sb[:], wm12_sb[:])
            ps_rm = ps.tile([P, F1], dt, tag="ps", name="ps")
            for k in range(KX):
                w = wmid.tile([P, F1], dt, tag="wmid", name="wmid")
                nc.sync.dma_start(w[:], w13_d[k])
                nc.tensor.matmul(
                    ps_rm[:], res1_sb[k][:], w[:],
                    start=(k == 0), stop=(k == KX - 1),
                )
            h1m_sb = work.tile([P, F1], dt, tag="h1m", name="h1m")
            nc.vector.tensor_tensor(h1m_sb[:], ps_rw[:], h1r_sb[:], op=OP.mult)
            h1f_sb = work.tile([P, F1], dt, tag="h1f", name="h1f")
            nc.vector.scalar_tensor_tensor(
                h1f_sb[:], ps_rm[:], 2e-4, h1m_sb[:], op0=OP.mult, op1=OP.add
            )

            # ---- res2 = (r@W2s) * h1f --------------------------------------
            ps_rw2 = ps.tile([P, F1], dt, tag="ps", name="ps")
            nc.tensor.matmul(ps_rw2[:], rT_sb[:], W2s_sb[:])
            res2_sb = work.tile([P, F1], dt, tag="res2", name="res2")
            nc.vector.tensor_tensor(
                res2_sb[:], ps_rw2[:], h1f_sb[:], op=OP.mult
            )

            # ---- GCN2 -------------------------------------------------------
            r2t = []
            for c2 in range(2):
                ps_tr = ps.tile([P, P], dt, tag="ps", name="ps")
                nc.tensor.transpose(
                    ps_tr[:], res2_sb[:, c2 * P:(c2 + 1) * P], ident[:]
                )
                t = work.tile([P, P], dt, tag=f"r2t{c2}", name=f"r2t{c2}")
                nc.vector.tensor_copy(t[:], ps_tr[:])
                r2t.append(t)
            g2W_sb = [load(wsml, [P, F2], g2W_d[k], "g2W") for k in range(2)]
            ps_h2 = ps.tile([P, F2], dt, tag="ps", name="ps")
            for c2 in range(2):
                nc.tensor.matmul(
                    ps_h2[:], r2t[c2][:], g2W_sb[c2][:],
                    start=(c2 == 0), stop=(c2 == 1),
                )
            h2g_sb = work.tile([P, F2], dt, tag="h2g", name="h2g")
            nc.scalar.activation(
                h2g_sb[:], ps_h2[:], AF.Copy, bias=0.0, scale=dinv_sb[:, 0:1]
            )
            cin2_t = dpool.tile([P, F2], dt, tag="cin2", name="cin2")
            cout2_t = dpool.tile([NNODE, F2], dt, tag="cout2", name="cout2")
            nc.sync.dma_start(cin2_t[:], h2g_sb[:])
            nc.gpsimd.collective_compute(
                "AllGather",
                OP.bypass,
                replica_groups=[list(range(NCORE))],
                ins=[cin2_t[:].opt()],
                outs=[cout2_t[:].opt()],
            )
            h2all = []
            for s in range(NCORE):
                t = hallp.tile([P, F2], dt, tag=f"h2all{s}", name=f"h2all{s}")
                nc.sync.dma_start(t[:], cout2_t[:][s * P:(s + 1) * P, :])
                h2all.append(t)
            ps_o2 = ps.tile([P, F2], dt, tag="ps", name="ps")
            for s in range(NCORE):
                nc.tensor.matmul(
                    ps_o2[:], A_sb[s][:], h2all[s][:],
                    start=(s == 0), stop=(s == NCORE - 1),
                )
            h2c_sb = work.tile([P, F2], dt, tag="h2c", name="h2c")
            nc.vector.scalar_tensor_tensor(
                h2c_sb[:], ps_o2[:], dinv_sb[:, 0:1], g2b_sb[:],
                op0=OP.mult, op1=OP.add,
            )
            h2r_sb = work.tile([P, F2], dt, tag="h2r", name="h2r")
            nc.scalar.activation(h2r_sb[:], h2c_sb[:], AF.Relu)

            # ---- final fc + log_softmax ------------------------------------
            ps_t2 = ps.tile([F2, P], dt, tag="ps", name="ps")
            nc.tensor.transpose(ps_t2[:], h2r_sb[:], ident[:])
            h2rT_sb = work.tile([F2, P], dt, tag="h2rT", name="h2rT")
            nc.vector.tensor_copy(h2rT_sb[:], ps_t2[:])
            ps_z = ps.tile([P, 8], dt, tag="ps", name="ps")
            nc.tensor.matmul(ps_z[:], h2rT_sb[:], fcW_sb[:])
            z_sb = work.tile([P, 8], dt, tag="z", name="z")
            nc.vector.tensor_tensor(z_sb[:], ps_z[:], fcb_sb[:], op=OP.add)
            mx2_sb = work.tile([P, 1], dt, tag="mx2", name="mx2")
            nc.vector.tensor_reduce(mx2_sb[:], z_sb[:], axis=AX.X, op=OP.max)
            nmx_sb = work.tile([P, 1], dt, tag="nmx", name="nmx")
            nc.vector.tensor_scalar_mul(nmx_sb[:], mx2_sb[:], -1.0)
            e_sb = work.tile([P, 8], dt, tag="e", name="e")
            ssum_sb = work.tile([P, 1], dt, tag="ssum", name="ssum")
            nc.scalar.activation(
                e_sb[:], z_sb[:], AF.Exp, bias=nmx_sb[:, 0:1], scale=1.0,
                accum_out=ssum_sb[:, 0:1],
            )
            lns_sb = work.tile([P, 1], dt, tag="lns", name="lns")
            nc.scalar.activation(lns_sb[:], ssum_sb[:], AF.Ln)
            c_sb = work.tile([P, 1], dt, tag="csum", name="csum")
            nc.vector.tensor_tensor(c_sb[:], mx2_sb[:], lns_sb[:], op=OP.add)
            o_sb = work.tile([P, 8], dt, tag="osb", name="osb")
            nc.vector.tensor_scalar(
                o_sb[:], z_sb[:], c_sb[:, 0:1], None, OP.subtract
            )
            nc.sync.dma_start(out_d[:, :], o_sb[:])

    nc.compile()
    return nc


def _get_module(T: int) -> "bacc.Bacc":
    if T not in _module_cache:
        _module_cache[T] = _build(T)
    return _module_cache[T]


def _f32c(a) -> np.ndarray:
    return np.ascontiguousarray(np.asarray(a, dtype=np.float32))


def _prepare(inputs):
    """Returns (T, in_maps)."""
    f = {k: np.asarray(v) for k, v in inputs.items()}
    x1, x11, x2 = f["x1"], f["x11"], f["x2"]
    edge = np.asarray(f["edge_index"]).astype(np.int64)
    src, dst = edge[0], edge[1]

    shared = {
        "mW1": _f32c(f["mlp_W1"].reshape(KX, P, H)),
        "mW2": _f32c(f["mlp_W2"].reshape(HT, P, H)),
        "mW3": _f32c(f["mlp_W3"].reshape(HT, P, 3)),
        "mb1": _f32c(np.asarray(f["mlp_b1"]).reshape(HT, P).T),
        "mb2": _f32c(np.asarray(f["mlp_b2"]).reshape(HT, P).T),
        "mb3": _f32c(np.broadcast_to(f["mlp_b3"], (P, 3))),
        "gW1": _f32c(f["m1_W1"].reshape(KX, P, H)),
        "gW2": _f32c(f["m1_W2"].reshape(HT, P, H)),
        "gW3": _f32c(f["m1_W3"].reshape(HT, P, 2)),
        "gb1": _f32c(np.asarray(f["m1_b1"]).reshape(HT, P).T),
        "gb2": _f32c(np.asarray(f["m1_b2"]).reshape(HT, P).T),
        "gb3": _f32c(np.asarray(f["m1_b3"]).reshape(2, 1)),
        "bp1": _f32c(f["bp1"]),
        "bp2": _f32c(f["bp2"]),
        "wm12": _f32c(f["wm12"]),
        "w13": _f32c(f["wm13"].reshape(KX, P, F1)),
        "g1W": _f32c(f["gcn1_W"].reshape(KX, P, F1)),
        "g1b": _f32c(np.broadcast_to(f["gcn1_b"], (P, F1))),
        "g2W": _f32c(f["gcn2_W"].reshape(2, P, F2)),
        "g2b": _f32c(np.broadcast_to(f["gcn2_b"], (P, F2))),
        "fcW": _f32c(f["fc_W"]),
        "fcb": _f32c(np.broadcast_to(f["fc_b"], (P, 8))),
    }

    csh = dst // P
    ssh = src // P
    srcl = (src % P).astype(np.float32)
    dstl = (dst % P).astype(np.float32)
    buckets = {}
    maxlen = 1
    for c in range(NCORE):
        for s in range(NCORE):
            m = (csh == c) & (ssh == s)
            buckets[(c, s)] = (srcl[m], dstl[m])
            maxlen = max(maxlen, int(m.sum()))
    T = (maxlen + P - 1) // P

    wm1 = _f32c(f["wm1"])
    cl2 = _f32c(f["class2"])
    wm2 = _f32c(f["wm2"])

    in_maps = []
    for c in range(NCORE):
        rows = slice(c * P, (c + 1) * P)
        k96 = slice(c * 96, (c + 1) * 96)
        k32 = slice(c * 32, (c + 1) * 32)
        es = np.full((NCORE * T, P), 255.0, np.float32)
        ed = np.full((NCORE * T, P), 255.0, np.float32)
        for s in range(NCORE):
            sl, dl = buckets[(c, s)]
            n = len(sl)
            es[s * T:(s + 1) * T].reshape(-1)[:n] = sl
            ed[s * T:(s + 1) * T].reshape(-1)[:n] = dl
        m = dict(shared)
        m["x1t"] = _f32c(x1[rows].T.reshape(KX, P, P))
        m["x11t"] = _f32c(x11[rows].T.reshape(KX, P, P))
        m["x2t"] = _f32c(x2[rows].T.reshape(KX, P, P))
        m["w1p"] = _f32c(wm1[:, k96, :].transpose(0, 2, 1).reshape(KX, P, 96))
        m["c2p"] = _f32c(cl2[:, k96, :].transpose(0, 2, 1))
        m["w2p"] = _f32c(wm2[:, k32, :].transpose(0, 2, 1).reshape(KX, P, 32))
        m["esrc"] = np.ascontiguousarray(es.T)
        m["edst"] = np.ascontiguousarray(ed.T)
        in_maps.append(m)
    return T, in_maps


def run(inputs, trace=False, **kw):
    """Full pipeline; returns (output [1024,8] f32, BassKernelResults)."""
    T, in_maps = _prepare(inputs)
    nc = _get_module(T)
    res = bass_utils.run_bass_kernel_spmd(
        nc, in_maps, core_ids=list(range(NCORE)), trace=trace, **kw
    )
    out = np.concatenate(
        [res.results[c]["out"] for c in range(NCORE)], axis=0
    ).astype(np.float32)
    return out, res


def kernel(**inputs) -> np.ndarray:
    out, _ = run(inputs)
    return out
